# revision 28
# baseline (speedup 1.0000x reference)
"""Trainium2 Bass kernel for nn_Attention_60739427500161.

Strategy (8 NeuronCores, head-sharded tensor parallel, no collectives):
- Core c handles batch c//4 and query heads 8g..8g+7 (g=c%4), i.e. kv heads
  2g,2g+1. Every core runs the IDENTICAL program on different data (no
  Switch arms, no cross-core skew). The out projection is computed as a
  partial product over the core's 512 gated features; the host sums the 4
  partials per batch during unshard (cheaper than an on-device all-reduce,
  whose entry barrier + latency previously cost more than it saved).
- vs the previous seq-sharded kernel this removes the 4x-duplicated k
  projection (131k -> 33k PE cycles) and the 4-way tc.Switch.
- x^T is streamed in 512-seq-column slabs (bufs=2) instead of held
  resident; the k projection is just a 9th projection tile of each slab.
- Attention runs per (strip s, head pair p): scoresT for head A (kT rows
  0-63, PE row-tile (0,0)) and head B (rows 64-127, row-tile (64,0)) are
  emitted back-to-back so the two 64-contraction matmuls can overlap on
  complementary halves of the PE array. exp on ACT (no max subtraction
  needed), causal diagonal handled by trimming the moving q-range per key
  chunk plus one gpsimd affine_select triangle per chunk.
- Denominator l via a 65th ones-column in kaug (the av stationary), so av
  needs no separate reduction. Gating computes av / (l * (1 + e^{-g})) with
  lrow copies + gpsimd partition broadcasts + 3 batched DVE ops per pair.
- The qg projection (next strip), k projection (next slab), kaug
  transposes and out projection (previous strip) are interleaved into the
  attention stream as PE filler so the TensorEngine never waits on the ACT
  exp pipeline and HAM stays warm.
- RoPE rotate_half as in the previous kernel: host permutes feature pairs
  (d, d+32) adjacent, DVE stream_shuffle([o^1]) produces the rotated
  operand, sin is sign-premultiplied on the host.
- All matmuls bf16; f32 accumulation in PSUM.
"""

import sys

for _p in ("/root/.axon_site/_ro/trn_rl_repo", "/opt/trn_rl_repo"):
    if _p not in sys.path:
        sys.path.append(_p)

import ml_dtypes
import numpy as np

import concourse.bass as bass
import concourse.mybir as mybir
import concourse.tile as tile
from concourse import bacc
from concourse.bass_utils import run_bass_kernel_spmd
from concourse.masks import make_identity

F32 = mybir.dt.float32
BF16 = mybir.dt.bfloat16
AF = mybir.ActivationFunctionType
ALU = mybir.AluOpType

B, S, HID = 2, 2048, 2048
NH, NKV, D = 32, 8, 64

# pi permutation: interleave (d, d+32) pairs so the rotate_half partner is
# the adjacent partition. pos(d) = 2d (d<32) else 2(d-32)+1.
_POS = np.array([2 * d if d < 32 else 2 * (d - 32) + 1 for d in range(D)])
_INV = np.argsort(_POS)
_SHUF_MASK = [o ^ 1 for o in range(32)]

_NC_CACHE = None
DEBUG_DUMPS = False


def _build_nc():
    nc = bacc.Bacc(None, target_bir_lowering=False)

    # All inputs are pre-laid-out on the host in SBUF image form ([128
    # partitions, cols]) so every load is a few big contiguous-row DMAs:
    # the Sync engine pays ~0.6us of descriptor-build per dma_start, so
    # trigger count (not bytes) dominated the old startup latency.
    xhd = nc.dram_tensor("xhd", [128, 4 * 16 * 512], BF16, kind="ExternalInput")
    wqd = nc.dram_tensor("wqd", [128, 8 * 16 * 128], BF16, kind="ExternalInput")
    wkd = nc.dram_tensor("wkd", [128, 16 * 128], BF16, kind="ExternalInput")
    wop = nc.dram_tensor("wop", [512, HID], BF16, kind="ExternalInput")
    cosp = nc.dram_tensor("cosp", [128, S], BF16, kind="ExternalInput")
    sinp = nc.dram_tensor("sinp", [128, S], BF16, kind="ExternalInput")
    outd = nc.dram_tensor("outd", [S, HID], BF16, kind="ExternalOutput")

    if DEBUG_DUMPS:
        dbg_qT = nc.dram_tensor("dbg_qT", [128, 8192], BF16, kind="ExternalOutput")
        dbg_sigT = nc.dram_tensor("dbg_sigT", [128, 8192], BF16, kind="ExternalOutput")
        dbg_kT = nc.dram_tensor("dbg_kT", [128, 2048], BF16, kind="ExternalOutput")
        dbg_kaug = nc.dram_tensor("dbg_kaug", [128, 2080], BF16, kind="ExternalOutput")
        dbg_gatedT = nc.dram_tensor("dbg_gatedT", [128, 8192], BF16, kind="ExternalOutput")
        dbg_et = nc.dram_tensor("dbg_et", [128, 1024], BF16, kind="ExternalOutput")
        dbg_av = nc.dram_tensor("dbg_av", [65, 1024], F32, kind="ExternalOutput")
        dbg_gt = nc.dram_tensor("dbg_gt", [128, 1536], F32, kind="ExternalOutput")

    with tile.TileContext(nc) as tc:
        with (
            tc.tile_pool(name="pers", bufs=1) as pers,
            tc.tile_pool(name="pX", bufs=2) as pX,
            tc.tile_pool(name="pRt", bufs=2) as pRt,
            tc.tile_pool(name="pET", bufs=5 if DEBUG_DUMPS else 6) as pET,
            tc.tile_pool(name="pGt", bufs=1) as pGt,
            tc.tile_pool(name="pOb", bufs=2 if DEBUG_DUMPS else 3) as pOb,
            tc.tile_pool(name="PS", bufs=1, space="PSUM") as PS,
        ):
            wq_sb = pers.tile([128, 8 * 16 * 128], BF16, tag="wq")
            wk_sb = pers.tile([128, 16 * 128], BF16, tag="wk")
            wo_sb = pers.tile([128, 4 * 2048], BF16, tag="wo")
            cos_sb = pers.tile([128, S], BF16, tag="cos")
            sin_sb = pers.tile([128, S], BF16, tag="sin")
            qT = pers.tile([128, 4 * 2048], BF16, tag="qT")
            sigT = pers.tile([128, 4 * 2048], BF16, tag="sigT")
            kT = pers.tile([128, 2048], BF16, tag="kT")
            kaug = pers.tile([128, 2 * 16 * 65], BF16, tag="kaug")
            gatedT = pers.tile([128, 4 * 2048], BF16, tag="gatedT")
            ident = pers.tile([128, 64], BF16, tag="ident")

            kaug4 = kaug[:].rearrange("p (h j d) -> p h j d", h=2, j=16)

            xs = [None] * 4

            def load_slab(s, ntrig=8):
                # slab s is contiguous in xhd; ntrig triggers spread queues
                xs[s] = pX.tile([128, 16 * 512], BF16, tag="xs", name="xs")
                w = 8192 // ntrig
                for h in range(ntrig):
                    nc.sync.dma_start(
                        xs[s][:, h * w : (h + 1) * w],
                        xhd[:, s * 8192 + h * w : s * 8192 + (h + 1) * w],
                    )

            # ---- static loads, critical path (slab0, wk, tables) first ----
            load_slab(0, ntrig=16)
            for h in range(2):
                nc.sync.dma_start(
                    wk_sb[:, h * 1024 : (h + 1) * 1024],
                    wkd[:, h * 1024 : (h + 1) * 1024],
                )
            nc.sync.dma_start(cos_sb[:], cosp[:])
            nc.sync.dma_start(sin_sb[:], sinp[:])
            make_identity(nc, ident[0:64, :])
            nc.sync.dma_start(ident[64:128, :], ident[0:64, :])
            nc.vector.memset(kaug4[:, :, :, 64:65], 1.0)
            for t in (0, 4, 1, 5, 2, 6, 3, 7):  # qg(0) consumption order
                nc.sync.dma_start(
                    wq_sb[:, t * 2048 : (t + 1) * 2048],
                    wqd[:, t * 2048 : (t + 1) * 2048],
                )
            for fc in range(4):
                nc.sync.dma_start(
                    wo_sb[:, fc * 2048 : (fc + 1) * 2048],
                    wop[fc * 128 : (fc + 1) * 128, :],
                )

            def rope_drain(ps, c0, out_ap):
                """out = ps*cos + shuffle(ps)*sin' over 512 seq cols at c0."""
                shf = pRt.tile([128, 512], F32, tag="shf", name="shf")
                nc.vector.stream_shuffle(shf[:], ps[:], _SHUF_MASK)
                t1 = pRt.tile([128, 512], F32, tag="t1", name="t1")
                nc.vector.tensor_tensor(t1[:], ps[:], cos_sb[:, c0 : c0 + 512], ALU.mult)
                t2 = pRt.tile([128, 512], F32, tag="t2", name="t2")
                nc.vector.tensor_tensor(t2[:], shf[:], sin_sb[:, c0 : c0 + 512], ALU.mult)
                nc.vector.tensor_tensor(out_ap, t1[:], t2[:], ALU.add)

            def emit_proj(kind, s, t=0):
                """One projection tile: 16 accumulating matmuls + drain.

                kind='k': k projection of slab s -> kT strip s
                kind='q': q pair-tile t of strip s -> qT (RoPE'd)
                kind='g': gate pair-tile t of strip s -> sigT (e^{-g})
                """
                ps = PS.tile([128, 512], F32, tag="proj", bufs=2, name="proj_ps")
                w = wk_sb if kind == "k" else wq_sb
                toff = 0 if kind == "k" else (t if kind == "q" else 4 + t) * 2048
                for kc in range(16):
                    nc.tensor.matmul(
                        ps[:],
                        w[:, toff + kc * 128 : toff + (kc + 1) * 128],
                        xs[s][:, kc * 512 : (kc + 1) * 512],
                        start=(kc == 0),
                        stop=(kc == 15),
                    )
                if kind == "k":
                    rope_drain(ps, s * 512, kT[:, s * 512 : (s + 1) * 512])
                elif kind == "q":
                    rope_drain(
                        ps, s * 512, qT[:, t * 2048 + s * 512 : t * 2048 + (s + 1) * 512]
                    )
                else:
                    nc.scalar.activation(
                        sigT[:, t * 2048 + s * 512 : t * 2048 + (s + 1) * 512],
                        ps[:],
                        AF.Exp,
                        scale=-1.0,
                    )

            def emit_transposes(s):
                """kaug chunks 4s..4s+3 for both local kv heads from kT."""
                for h in range(2):
                    tr = PS.tile([128, 256], BF16, tag="sc", bufs=2, name="tr")
                    for u in range(4):
                        nc.tensor.transpose(
                            tr[:, u * 64 : (u + 1) * 64],
                            kT[h * 64 : (h + 1) * 64, s * 512 + u * 128 : s * 512 + (u + 1) * 128],
                            ident[h * 64 : h * 64 + 64, :],
                        )
                    nc.vector.tensor_copy(
                        kaug4[:, h, 4 * s : 4 * s + 4, 0:64],
                        tr[:].rearrange("p (u d) -> p u d", u=4),
                    )

            def emit_gating(s, p, avA, avB):
                """gatedT = av / (l * (1 + e^{-g})) for pair p, strip s.

                First evacuate both av banks to SBUF with two DVE copies so
                the next pair's av matmuls aren't stalled behind the whole
                gating chain (PSUM ring WAR)."""
                avs = pGt.tile([65, 1024], F32, tag="avs", name="avs")
                nc.vector.tensor_copy(avs[:, 0:512], avA[:])
                nc.vector.tensor_copy(avs[:, 512:1024], avB[:])
                avA, avB = avs[:, 0:512], avs[:, 512:1024]
                lrA = pGt.tile([1, 512], F32, tag="lrA", name="lrA")
                nc.scalar.copy(lrA[:], avA[64:65, :])
                lrB = pGt.tile([1, 512], F32, tag="lrB", name="lrB")
                nc.scalar.copy(lrB[:], avB[64:65, :])
                # two base-0 broadcast targets: partition_broadcast ignores a
                # nonzero out base partition, so a [128,512] lb can't be
                # filled half-and-half.
                lbA = pGt.tile([64, 512], F32, tag="lbA", name="lbA")
                nc.gpsimd.partition_broadcast(lbA[:], lrA[:])
                lbB = pGt.tile([64, 512], F32, tag="lbB", name="lbB")
                nc.gpsimd.partition_broadcast(lbB[:], lrB[:])
                egA = sigT[0:64, p * 2048 + s * 512 : p * 2048 + (s + 1) * 512]
                egB = sigT[64:128, p * 2048 + s * 512 : p * 2048 + (s + 1) * 512]
                egBc = pGt.tile([64, 512], F32, tag="egBc", name="egBc")
                nc.vector.tensor_copy(egBc[:], egB)
                denA = pGt.tile([64, 512], F32, tag="denA", name="denA")
                nc.vector.scalar_tensor_tensor(denA[:], egA, 1.0, lbA[:], ALU.add, ALU.mult)
                denB = pGt.tile([64, 512], F32, tag="denB", name="denB")
                nc.vector.scalar_tensor_tensor(denB[:], egBc[:], 1.0, lbB[:], ALU.add, ALU.mult)
                rdenA = pGt.tile([64, 512], F32, tag="rdenA", name="rdenA")
                nc.vector.reciprocal_approx_fast(rdenA[:], denA[:])
                rdenB = pGt.tile([64, 512], F32, tag="rdenB", name="rdenB")
                nc.vector.reciprocal_approx_fast(rdenB[:], denB[:])
                if DEBUG_DUMPS and s == 0 and p == 0:
                    av_sb = pGt.tile([65, 1024], F32, tag="avsb", bufs=1, name="av_sb")
                    nc.scalar.copy(av_sb[:, 0:512], avA[:])
                    nc.scalar.copy(av_sb[:, 512:1024], avB[:])
                    nc.sync.dma_start(dbg_av[:], av_sb[:])
                    nc.sync.dma_start(dbg_gt[0:64, 0:512], lbA[:])
                    nc.sync.dma_start(dbg_gt[64:128, 0:512], lbB[:])
                    nc.sync.dma_start(dbg_gt[0:64, 512:1024], denA[:])
                    nc.sync.dma_start(dbg_gt[64:128, 512:1024], denB[:])
                    nc.sync.dma_start(dbg_gt[0:64, 1024:1536], rdenA[:])
                    nc.sync.dma_start(dbg_gt[64:128, 1024:1536], rdenB[:])
                rowA = (p % 2) * 64
                fcA, fcB = p // 2, 2 + p // 2
                nc.vector.tensor_tensor(
                    gatedT[rowA : rowA + 64, fcA * 2048 + s * 512 : fcA * 2048 + (s + 1) * 512],
                    avA[0:64, :],
                    rdenA[:],
                    ALU.mult,
                )
                nc.vector.tensor_tensor(
                    gatedT[rowA : rowA + 64, fcB * 2048 + s * 512 : fcB * 2048 + (s + 1) * 512],
                    avB[0:64, :],
                    rdenB[:],
                    ALU.mult,
                )

            def emit_out_unit(s, mi):
                """out partial rows [s*512+mi*128, +128) x all 2048 cols.

                16 MMs over (oc, fc), staged into one bf16 row-block so the
                write back is a single 4KB-row DMA."""
                ob = pOb.tile([128, 2048], BF16, tag="ob", name="ob")
                for oc in range(4):
                    op = PS.tile([128, 512], F32, tag="proj", bufs=2, name="op_ps")
                    for fc in range(4):
                        nc.tensor.matmul(
                            op[:],
                            gatedT[:, fc * 2048 + s * 512 + mi * 128 : fc * 2048 + s * 512 + (mi + 1) * 128],
                            wo_sb[:, fc * 2048 + oc * 512 : fc * 2048 + (oc + 1) * 512],
                            start=(fc == 0),
                            stop=(fc == 3),
                        )
                    nc.vector.tensor_copy(ob[:, oc * 512 : (oc + 1) * 512], op[:])
                nc.sync.dma_start(
                    outd[s * 512 + mi * 128 : s * 512 + (mi + 1) * 128, :], ob[:]
                )

            def emit_attention(s, p, fillers, fidx, fend):
                """scoresT/exp/mask/av/gating for pair p, strip s.

                fillers[fidx:fend] are emitted evenly between av groups."""
                n = (s + 1) * 4
                avA = PS.tile([65, 512], F32, tag="av", bufs=2, name="avA")
                avB = PS.tile([65, 512], F32, tag="av", bufs=2, name="avB")
                qA = qT[0:64, p * 2048 + s * 512 : p * 2048 + (s + 1) * 512]
                qB = qT[64:128, p * 2048 + s * 512 : p * 2048 + (s + 1) * 512]
                ngroups = n // 2
                emitted = 0
                pend = []  # older groups' (j, q0, et2): avs lag two groups
                            # so exp + affine deps are long satisfied

                def flush_avs(keep=0):
                    while len(pend) > keep:
                        j, q0, et2 = pend.pop(0)
                        nc.tensor.matmul(
                            avA[0:65, q0:512],
                            kaug4[:, 0, j, :],
                            et2[:, 0, q0:512],
                            start=(j == 0),
                            stop=(j == n - 1),
                            skip_group_check=True,
                        )
                        nc.tensor.matmul(
                            avB[0:65, q0:512],
                            kaug4[:, 1, j, :],
                            et2[:, 1, q0:512],
                            start=(j == 0),
                            stop=(j == n - 1),
                            skip_group_check=True,
                        )

                for grp in range(ngroups):
                    ets = []
                    for j in (2 * grp, 2 * grp + 1):
                        q0 = max(0, (j - 4 * s) * 128)
                        sc = PS.tile([128, 1024], F32, tag="sc", bufs=2, name="sc")
                        sc2 = sc[:].rearrange("p (u q) -> p u q", u=2)
                        nc.tensor.matmul(
                            sc2[:, 0, q0:512],
                            kT[0:64, j * 128 : (j + 1) * 128],
                            qA[:, q0:512],
                            start=True,
                            stop=True,
                        )
                        nc.tensor.matmul(
                            sc2[:, 1, q0:512],
                            kT[64:128, j * 128 : (j + 1) * 128],
                            qB[:, q0:512],
                            start=True,
                            stop=True,
                        )
                        et = pET.tile([128, 1024], BF16, tag="et", name="et")
                        et2 = et[:].rearrange("p (u q) -> p u q", u=2)
                        nc.scalar.activation(et2[:, :, q0:512], sc2[:, :, q0:512], AF.Exp)
                        if j >= 4 * s:
                            # causal triangle on the leading 128 q-cols
                            nc.gpsimd.affine_select(
                                et2[:, :, q0 : q0 + 128],
                                et2[:, :, q0 : q0 + 128],
                                pattern=[[0, 2], [1, 128]],
                                compare_op=ALU.is_ge,
                                fill=0.0,
                                base=0,
                                channel_multiplier=-1,
                            )
                        if DEBUG_DUMPS and s == 0 and p == 0 and j == 0:
                            nc.sync.dma_start(dbg_et[:], et[:])
                        ets.append((j, q0, et2))
                    pend.extend(ets)
                    flush_avs(keep=2)
                    want = (fend - fidx) * (grp + 1) // ngroups
                    while emitted < want:
                        fillers[fidx + emitted]()
                        emitted += 1
                flush_avs()
                emit_gating(s, p, avA, avB)

            # ---- prelude: k(0), kaug(0), qg(0)  (slab 0 already loading) ----
            emit_proj("k", 0)
            emit_transposes(0)
            load_slab(1)
            for t in range(4):
                emit_proj("q", 0, t)
                emit_proj("g", 0, t)

            # ---- strip loop ----
            for s in range(4):
                if s + 2 < 4:
                    load_slab(s + 2)
                fillers = []
                if s + 1 < 4:
                    fillers.append(lambda s=s: emit_proj("k", s + 1))
                    fillers.append(lambda s=s: emit_transposes(s + 1))
                    for t in range(4):
                        fillers.append(lambda s=s, t=t: emit_proj("q", s + 1, t))
                        fillers.append(lambda s=s, t=t: emit_proj("g", s + 1, t))
                if s >= 1:
                    n_inline = 2 if s == 3 else 4  # save 2 units for the
                    for mi in range(n_inline):     # strip-3 gating bubble
                        fillers.append(lambda s=s, mi=mi: emit_out_unit(s - 1, mi))
                nf = len(fillers)
                for p in range(4):
                    f0 = nf * p // 4
                    f1 = nf * (p + 1) // 4
                    emit_attention(s, p, fillers, f0, f1)

            # ---- tail: out(2) leftovers fill the gating chain, then out(3) ----
            for mi in (2, 3):
                emit_out_unit(2, mi)
            for mi in range(4):
                emit_out_unit(3, mi)

            if DEBUG_DUMPS:
                nc.sync.dma_start(dbg_qT[:], qT[:])
                nc.sync.dma_start(dbg_sigT[:], sigT[:])
                nc.sync.dma_start(dbg_kT[:], kT[:])
                nc.sync.dma_start(dbg_kaug[:], kaug[:])
                nc.sync.dma_start(dbg_gatedT[:], gatedT[:])

    nc.compile()
    return nc


def _get_nc():
    global _NC_CACHE
    if _NC_CACHE is None:
        _NC_CACHE = _build_nc()
    return _NC_CACHE


def _sbuf_image(m):
    """[HID, C] contraction-major matrix -> SBUF image [128, 16*C]
    (partition p, col kc*C+i) = m[kc*128+p, i], flattened row-major."""
    hid, c = m.shape
    return np.ascontiguousarray(
        m.reshape(16, 128, c).transpose(1, 0, 2).reshape(128, 16 * c)
    )


def _prep_inputs(hidden_states, cos, sin, wq, wk, wo):
    """Build the 8 per-core input maps (host-side slicing/permutation).

    Everything is pre-laid-out as SBUF images so device loads are big
    contiguous-row DMAs."""
    inv = _INV
    dmap = np.concatenate([inv, inv])
    sign = np.where((np.arange(128) % 64) % 2 == 0, -1.0, 1.0).astype(np.float32)

    cosT = np.ascontiguousarray(cos[:, dmap].T).astype(ml_dtypes.bfloat16)
    sinT = np.ascontiguousarray(sin[:, dmap].T * sign[:, None]).astype(
        ml_dtypes.bfloat16
    )

    wq_q = wq[:, : NH * D]
    wq_g = wq[:, NH * D :]

    # x slab-major: [p, s*8192 + kc*512 + i] = x[kc*128+p, s*512+i]
    xhs = []
    for b in range(B):
        xT = hidden_states[b].T  # [HID, S]
        xh = (
            xT.reshape(16, 128, 4, 512)
            .transpose(1, 2, 0, 3)
            .reshape(128, 4 * 16 * 512)
        )
        xhs.append(np.ascontiguousarray(xh).astype(ml_dtypes.bfloat16))

    per_group = []
    for g in range(4):
        tiles = []
        for p in range(4):
            hA, hB = 8 * g + p, 8 * g + 4 + p
            tiles.append(
                np.concatenate([wq_q[:, hA * D + inv], wq_q[:, hB * D + inv]], axis=1)
            )
        for p in range(4):
            hA, hB = 8 * g + p, 8 * g + 4 + p
            tiles.append(
                np.concatenate([wq_g[:, hA * D + inv], wq_g[:, hB * D + inv]], axis=1)
            )
        wqd = np.concatenate([_sbuf_image(t) for t in tiles], axis=1).astype(
            ml_dtypes.bfloat16
        )  # [128, 8*2048]
        wkd = _sbuf_image(
            np.concatenate(
                [wk[:, (2 * g) * D + inv], wk[:, (2 * g + 1) * D + inv]], axis=1
            )
        ).astype(ml_dtypes.bfloat16)  # [128, 2048]
        worows = []
        for fc in range(4):
            worows.append(wo[(8 * g + 2 * fc) * D + inv, :])
            worows.append(wo[(8 * g + 2 * fc + 1) * D + inv, :])
        wop = np.ascontiguousarray(np.concatenate(worows, axis=0)).astype(
            ml_dtypes.bfloat16
        )  # [512, HID]
        per_group.append((wqd, wkd, wop))

    in_maps = []
    for c in range(8):
        b, g = c // 4, c % 4
        wqd, wkd, wop = per_group[g]
        in_maps.append(
            {
                "xhd": xhs[b],
                "wqd": wqd,
                "wkd": wkd,
                "wop": wop,
                "cosp": cosT,
                "sinp": sinT,
            }
        )
    return in_maps


def kernel(hidden_states, cos, sin, attention_mask, wq, wk, wv, wo, **_unused):
    hidden_states = np.asarray(hidden_states, dtype=np.float32)
    cos = np.asarray(cos, dtype=np.float32)
    sin = np.asarray(sin, dtype=np.float32)
    wq = np.asarray(wq, dtype=np.float32)
    wk = np.asarray(wk, dtype=np.float32)
    wo = np.asarray(wo, dtype=np.float32)

    nc = _get_nc()
    in_maps = _prep_inputs(hidden_states, cos, sin, wq, wk, wo)
    res = run_bass_kernel_spmd(nc, in_maps, core_ids=list(range(8)))

    y = np.zeros((B, S, HID), dtype=np.float32)
    for c in range(8):
        y[c // 4] += np.asarray(res.results[c]["outd"], dtype=np.float32)
    return y
